# revision 50
# baseline (speedup 1.0000x reference)
"""MultiHeadEMA Trainium2 Bass kernel (radix-16 blocked scan, matmul-offloaded).

Reference computation (B=4, S=8192, D=1024, N=2):
    out = silu(conv_causal(x, k) + x * omega)
    k[d, l] = sum_n c[d, n] * q[d, n]^l
    q = 1 - sigmoid(delta) * sigmoid(alpha)
    c = sigmoid(delta) * beta * gamma * sqrt(1/N)

The causal conv is a pair of first-order recurrences per channel.  On TRN2
the per-partition-scalar DVE ops (scalar_tensor_tensor / tensor_tensor_scan)
run at 1 elem/cycle with no fast modes, so a pure-Vector implementation is
Vector-bound.  This version blocks the recurrence by J=16 timesteps and
restructures all muladd work as cross-partition matmuls on the otherwise
idle Tensor engine:

  - layout: partition p = (phase j in [0,16), channel c in [0,8)) per group
    of 8 channels; 16 groups cover the core's 128 channels; free dim is the
    block index m in [0, M=S/16).
  - u[m]    = sum_j q^{15-j} x[16m+j]        -> matmul pass (weights wu)
  - h[m]    = q^16 h[m-1] + u[m]             -> DVE scan, 16x shorter, both
              recurrences x 8 groups stacked per 128-partition scan
  - y[16m+j] = sum_n c_n q_n^{j+1} h_n[m-1]  -> matmul pass (weights wv)
             + sum_{i<=j} g_{j-i} x[16m+i] + w x[16m+j]  -> matmul pass (wx)
  - out = silu(y) fused on the Scalar engine, PSUM -> fp16 SBUF.

Groups pair up (2k, 2k+1) so u-matmuls accumulate into a shared 32-partition
PSUM window (tile_position must be 32-aligned) and each ACT covers a
1024-wide y pair.  fp16 end-to-end I/O halves DMA traffic (8 MiB in + 8 MiB
out per core); weights are host-computed in fp64 and shipped as fp16; the
scan multiplier q^16 and scan state stay fp32.  Numpy sim of this exact
quantization measures rel err ~6e-4 vs the fp32 reference.

Sharding: D=1024 split across 8 cores (128 channels each); host packs the
phase-major fp16 layout and unpacks the result (part of shard/unshard).
"""

import math

import numpy as np

import concourse.bass as bass
import concourse.mybir as mybir
import concourse.tile as tile
from concourse import bacc
from concourse.bass_utils import run_bass_kernel_spmd

B = 4
S = 8192
D = 1024
N = 2
N_CORES = 8
D_LOC = D // N_CORES      # 128 channels per core
J = 16                    # timesteps per block (radix)
C = 8                     # channels per group
G = D_LOC // C            # 16 groups
M = S // J                # 512 blocks per batch
SCALE = math.sqrt(1.0 / N)

F32 = mybir.dt.float32
F16 = mybir.dt.float16


def build_nc(x_bufs=4, o_bufs=3, h_bufs=3, u_bufs=1, y_bufs=1, act="Silu"):
    """Per-core Bass module (SPMD: same NEFF on all cores).

    Inputs (per core):
      x  [B, 128, G*M] f16 — phase-major shard: x[b, j*8+c, g*M+m]
                              = x_orig[b, t=16m+j, ch=8g+c]
      wu [128, G*32]   f16 — u-prep weights, lhsT per group
      wv [128, G*128]  f16 — h-combine weights, lhsT per group
      wx [128, G*128]  f16 — x-combine weights, lhsT per group
      a  [128, 2]      f32 — scan multipliers q^16 for stacked tiles A, B
    Output:
      o  [B, 128, G*M] f16 — same layout as x
    """
    nc = bacc.Bacc(
        "TRN2",
        target_bir_lowering=False,
        debug=False,
        enable_asserts=False,
        num_devices=N_CORES,
    )

    x_d = nc.dram_tensor("x", [B, 128, G * M], F16, kind="ExternalInput").ap()
    wu_d = nc.dram_tensor("wu", [128, G * 32], F16, kind="ExternalInput").ap()
    wv_d = nc.dram_tensor("wv", [128, G * 128], F16, kind="ExternalInput").ap()
    wx_d = nc.dram_tensor("wx", [128, G * 128], F16, kind="ExternalInput").ap()
    a_d = nc.dram_tensor("a", [128, 2], F32, kind="ExternalInput").ap()
    o_d = nc.dram_tensor("o", [B, 128, G * M], F16, kind="ExternalOutput").ap()

    mult = mybir.AluOpType.mult
    add = mybir.AluOpType.add
    ACT = getattr(mybir.ActivationFunctionType, act)
    HALF = G * M // 2

    with tile.TileContext(nc) as tc:
        with (
            tc.tile_pool(name="w", bufs=1) as w_pool,
            tc.tile_pool(name="x", bufs=x_bufs) as x_pool,
            tc.tile_pool(name="o", bufs=o_bufs) as o_pool,
            tc.tile_pool(name="h", bufs=h_bufs) as h_pool,
            tc.tile_pool(name="u", bufs=u_bufs, space="PSUM") as u_pool,
            tc.tile_pool(name="y", bufs=y_bufs, space="PSUM") as y_pool,
        ):
            # param DMAs go FIRST on the Sync ring: DMA-completion semaphore
            # thresholds aggregate per lane in schedule order, so anything
            # issued after a big x transfer inherits its completion time.
            # The scan's `a` multiplier must not wait on x batches.
            wu_t = w_pool.tile([128, G * 32], F16, tag="wu")
            wv_t = w_pool.tile([128, G * 128], F16, tag="wv")
            wx_t = w_pool.tile([128, G * 128], F16, tag="wx")
            a_t = w_pool.tile([128, 2], F32, tag="a")
            # issue order tunes both sem-lane thresholds and arrival time:
            # tiny scan/u params first, then batch-0 x in quarters (so the
            # first u-matmuls start ~5us earlier), then the combine weights
            # (first needed ~15us in), then the remaining batches.
            # tiny scan/u params ride the Scalar ring (issues in parallel
            # with Sync's x0 chunk; nothing big ever queues on Scalar's ring
            # so their completion thresholds clear immediately)
            nc.scalar.dma_start(out=a_t[:], in_=a_d[:])
            nc.scalar.dma_start(out=wu_t[:], in_=wu_d[:])
            x_tiles = [x_pool.tile([128, G * M], F16, tag="x", name=f"x{b}")
                       for b in range(B)]
            QTR = G * M // 4
            # x0's first half in quarters: the tile-A u-matmuls for groups
            # 0-3 start as soon as the first 512KB lands
            nc.sync.dma_start(out=x_tiles[0][:, :QTR], in_=x_d[0, :, :QTR])
            nc.sync.dma_start(
                out=x_tiles[0][:, QTR:HALF], in_=x_d[0, :, QTR:HALF]
            )
            # combine weights land before x0's tail so the first combines
            # start right after scan A of batch 0
            nc.sync.dma_start(out=wv_t[:], in_=wv_d[:])
            nc.sync.dma_start(out=wx_t[:], in_=wx_d[:])
            nc.sync.dma_start(out=x_tiles[0][:, HALF:], in_=x_d[0, :, HALF:])
            # bulk loads in quarters: finer interleaving with the output
            # ships on the shared DMA queues (2MB transfers measured worse)
            for b in range(1, B):
                for q in range(4):
                    nc.sync.dma_start(
                        out=x_tiles[b][:, q * QTR : (q + 1) * QTR],
                        in_=x_d[b, :, q * QTR : (q + 1) * QTR],
                    )

            # PE executes matmuls strictly in emission order, so batch b+1's
            # u-prep must be emitted BETWEEN batch b's combine pairs (once
            # x(b+1) has landed) or the ACT stream stalls at every batch
            # boundary waiting for the next scans.
            u_tiles = {}
            h_tiles = {}

            def emit_u(b, tidx, ks=None):
                """u-prep matmuls for stacked tile tidx (A: groups 0-7)."""
                u_tiles.setdefault(b, [None, None])
                if u_tiles[b][tidx] is None:
                    u_tiles[b][tidx] = u_pool.tile(
                        [128, M], F32, tag=f"u{tidx}", name=f"u{tidx}_{b}"
                    )
                xb = x_tiles[b]
                for k in ks if ks is not None else range(tidx * 4, tidx * 4 + 4):
                    w = k % 4
                    for half in range(2):
                        g = 2 * k + half
                        nc.tensor.matmul(
                            u_tiles[b][tidx][w * 32 : w * 32 + 32, :],
                            lhsT=wu_t[:, g * 32 : (g + 1) * 32],
                            rhs=xb[:, g * M : (g + 1) * M],
                            start=(half == 0),
                            stop=(half == 1),
                            tile_position=(0, w * 32),
                        )

            def emit_scan(b, tidx):
                """h[m] = q^16 h[m-1] + u[m]; fp32 state, fp16 stored h;
                col 0 holds h[-1] = 0 (batches are independent)."""
                h_tiles.setdefault(b, [None, None])
                ht = h_pool.tile(
                    [128, M + 1], F16, tag=f"h{tidx}", name=f"h{tidx}_{b}"
                )
                h_tiles[b][tidx] = ht
                nc.vector.memset(ht[:, 0:1], 0.0)
                nc.vector.tensor_tensor_scan(
                    ht[:, 1 : M + 1],
                    a_t[:, tidx : tidx + 1].broadcast_to([128, M]),
                    u_tiles[b][tidx][:, :],
                    0.0,
                    mult,
                    add,
                )

            def emit_combine(b, g0, ng, ob, y):
                """V+X matmuls for groups [g0, g0+ng) into y, then one ACT."""
                xb = x_tiles[b]
                for i in range(ng):
                    g = g0 + i
                    hv = h_tiles[b][g // 8]
                    ys = y[:, i * M : (i + 1) * M]
                    # x-term first: it only needs xb, so the PE can run it
                    # before the scan (which gates the h-term) completes.
                    nc.tensor.matmul(
                        ys,
                        lhsT=wx_t[:, g * 128 : (g + 1) * 128],
                        rhs=xb[:, g * M : (g + 1) * M],
                        start=True,
                        stop=False,
                    )
                    nc.tensor.matmul(
                        ys,
                        lhsT=wv_t[:, g * 128 : (g + 1) * 128],
                        rhs=hv[:, 0:M],
                        start=False,
                        stop=True,
                    )
                nc.scalar.activation(ob[:, g0 * M : (g0 + ng) * M], y[:], ACT)

            emit_u(0, 0)
            emit_scan(0, 0)
            emit_u(0, 1)
            emit_scan(0, 1)
            # Per batch the 16 groups run as 3|3|3|3|2|2 ACT blocks between
            # two strictly-alternating 3-bank PSUM tags: matmuls fill one
            # tile while ScalarE drains the other, and the bigger FD
            # amortizes the ~300-cycle per-ACT overhead.
            # The 16 groups per batch run as 3|3|2|2|2|2|2 ACT blocks.  The
            # y tags alternate via a GLOBAL counter and the block count is
            # odd, so a block's tag was always freed two ACTs ago, including
            # across batch boundaries.
            BLOCKS = [(0, 3), (3, 3), (6, 2), (8, 2), (10, 2), (12, 2), (14, 2)]
            EIGHTH = G * M // 8
            yslot = 0
            for b in range(B):
                ob = o_pool.tile([128, G * M], F16, tag="o", name=f"o{b}")
                for bi, (g0, ng) in enumerate(BLOCKS):
                    ytag = f"y{yslot % 2}"
                    yslot += 1
                    y = y_pool.tile(
                        [128, 3 * M], F32, tag=ytag, name=f"{ytag}_{b}_{g0}"
                    )
                    emit_combine(b, g0, ng, ob, y[:, : ng * M])
                    # prefetch next batch's u/scans between blocks
                    if b + 1 < B:
                        if bi == 1:
                            emit_u(b + 1, 0)
                            emit_scan(b + 1, 0)
                        elif bi == 3:
                            emit_u(b + 1, 1)
                            emit_scan(b + 1, 1)
                    # ship finished output spans on the Sync ring (ScalarE
                    # stays pure ACT); finer chunks on the last batch to
                    # shrink the drain tail.
                    if b < B - 1:
                        ship = {3: (0, HALF), 6: (HALF, G * M)}.get(bi)
                    else:
                        ship = {1: (0, QTR), 3: (QTR, 2 * QTR),
                                4: (2 * QTR, 3 * QTR),
                                5: (6 * EIGHTH, 7 * EIGHTH),
                                6: (7 * EIGHTH, 8 * EIGHTH)}.get(bi)
                    if ship is not None:
                        nc.sync.dma_start(
                            out=o_d[b, :, ship[0] : ship[1]],
                            in_=ob[:, ship[0] : ship[1]],
                        )

    nc.compile()
    return nc


def _host_params(delta, alpha, beta, gamma, omega, sl):
    """Per-core weight construction (channel slice sl; fp64 math)."""
    d = delta[sl, :, 0].astype(np.float64)
    al = alpha[sl, :, 0].astype(np.float64)
    p = 1.0 / (1.0 + np.exp(-d))
    aa = 1.0 / (1.0 + np.exp(-al))
    q = 1.0 - p * aa                                     # [128, N]
    c = p * beta[sl, :, 0].astype(np.float64) * gamma[sl].astype(np.float64) * SCALE
    w = omega[sl].astype(np.float64)                     # [128]
    ch = np.arange(D_LOC).reshape(G, C)                  # ch[g, cc] = 8g+cc

    qp = q[:, :, None] ** np.arange(J + 2)[None, None, :]   # [128, N, J+2]
    g_r = np.einsum("dn,dnr->dr", c, qp)                 # [128, J+2]

    # stacked-tile partition of (g, n, cc):
    #   tile = g//8, w = (g%8)//2, half = g%2 -> p = 32w + 16*half + 8n + cc
    def stack_p(g, n, cc):
        rem = g % 8
        return 32 * (rem // 2) + 16 * (g % 2) + 8 * n + cc

    wu = np.zeros((G, 128, 32))
    wv = np.zeros((G, 128, 128))
    wx = np.zeros((G, 128, 128))
    a = np.zeros((128, 2))
    for g in range(G):
        for cc in range(C):
            d_ = ch[g, cc]
            # wu cols are window-relative: 16*half + 8n + cc
            col0 = 16 * (g % 2)
            for n in range(N):
                a[stack_p(g, n, cc), g // 8] = qp[d_, n, J]
                for j in range(J):
                    wu[g, j * C + cc, col0 + 8 * n + cc] = qp[d_, n, J - 1 - j]
                    wv[g, stack_p(g, n, cc), j * C + cc] = c[d_, n] * qp[d_, n, j + 1]
            for j in range(J):
                for i in range(j + 1):
                    wx[g, i * C + cc, j * C + cc] = g_r[d_, j - i] + (
                        w[d_] if i == j else 0.0
                    )

    return (
        np.ascontiguousarray(wu.transpose(1, 0, 2).reshape(128, G * 32)).astype(np.float16),
        np.ascontiguousarray(wv.transpose(1, 0, 2).reshape(128, G * 128)).astype(np.float16),
        np.ascontiguousarray(wx.transpose(1, 0, 2).reshape(128, G * 128)).astype(np.float16),
        a.astype(np.float32),
    )


_NC_CACHE = {}


def kernel(x, delta, alpha, beta, gamma, omega):
    x = np.asarray(x, dtype=np.float32)
    delta = np.asarray(delta, dtype=np.float32)
    alpha = np.asarray(alpha, dtype=np.float32)
    beta = np.asarray(beta, dtype=np.float32)
    gamma = np.asarray(gamma, dtype=np.float32)
    omega = np.asarray(omega, dtype=np.float32)
    assert x.shape == (B, S, D)

    if "nc" not in _NC_CACHE:
        _NC_CACHE["nc"] = build_nc()
    nc = _NC_CACHE["nc"]

    xt = x.transpose(0, 2, 1)  # [B, D, S]
    in_maps = []
    for i in range(N_CORES):
        sl = slice(i * D_LOC, (i + 1) * D_LOC)
        wu, wv, wx, a = _host_params(delta, alpha, beta, gamma, omega, sl)
        # phase-major pack: [B, 128ch, S] -> [B, (j,c), g*M+m]
        xs = xt[:, sl, :].reshape(B, G, C, M, J)
        x_dev = np.ascontiguousarray(
            xs.transpose(0, 4, 2, 1, 3).reshape(B, 128, G * M)
        ).astype(np.float16)
        in_maps.append({"x": x_dev, "wu": wu, "wv": wv, "wx": wx, "a": a})

    res = run_bass_kernel_spmd(nc, in_maps, core_ids=list(range(N_CORES)))

    out = np.empty((B, S, D), dtype=np.float32)
    for i in range(N_CORES):
        sl = slice(i * D_LOC, (i + 1) * D_LOC)
        ov = res.results[i]["o"].reshape(B, J, C, G, M)
        oc = ov.transpose(0, 3, 2, 4, 1).reshape(B, D_LOC, S)  # [b, ch, t]
        out[:, :, sl] = oc.transpose(0, 2, 1).astype(np.float32)
    return out


# revision 51
# speedup vs baseline: 1.0874x; 1.0874x over previous
"""MultiHeadEMA Trainium2 Bass kernel (radix-16 blocked scan, matmul-offloaded).

Reference computation (B=4, S=8192, D=1024, N=2):
    out = silu(conv_causal(x, k) + x * omega)
    k[d, l] = sum_n c[d, n] * q[d, n]^l
    q = 1 - sigmoid(delta) * sigmoid(alpha)
    c = sigmoid(delta) * beta * gamma * sqrt(1/N)

The causal conv is a pair of first-order recurrences per channel.  On TRN2
the per-partition-scalar DVE ops (scalar_tensor_tensor / tensor_tensor_scan)
run at 1 elem/cycle with no fast modes, so a pure-Vector implementation is
Vector-bound.  This version blocks the recurrence by J=16 timesteps and
restructures all muladd work as cross-partition matmuls on the otherwise
idle Tensor engine:

  - layout: partition p = (phase j in [0,16), channel c in [0,8)) per group
    of 8 channels; 16 groups cover the core's 128 channels; free dim is the
    block index m in [0, M=S/16).
  - u[m]    = sum_j q^{15-j} x[16m+j]        -> matmul pass (weights wu)
  - h[m]    = q^16 h[m-1] + u[m]             -> DVE scan, 16x shorter, both
              recurrences x 8 groups stacked per 128-partition scan
  - y[16m+j] = sum_n c_n q_n^{j+1} h_n[m-1]  -> matmul pass (weights wv)
             + sum_{i<=j} g_{j-i} x[16m+i] + w x[16m+j]  -> matmul pass (wx)
  - out = silu(y) fused on the Scalar engine, PSUM -> fp16 SBUF.

Groups pair up (2k, 2k+1) so u-matmuls accumulate into a shared 32-partition
PSUM window (tile_position must be 32-aligned) and each ACT covers a
1024-wide y pair.  fp16 end-to-end I/O halves DMA traffic (8 MiB in + 8 MiB
out per core); weights are host-computed in fp64 and shipped as fp16; the
scan multiplier q^16 and scan state stay fp32.  Numpy sim of this exact
quantization measures rel err ~6e-4 vs the fp32 reference.

Sharding: D=1024 split across 8 cores (128 channels each); host packs the
phase-major fp16 layout and unpacks the result (part of shard/unshard).
"""

import math

import numpy as np

import concourse.bass as bass
import concourse.mybir as mybir
import concourse.tile as tile
from concourse import bacc
from concourse.bass_utils import run_bass_kernel_spmd

B = 4
S = 8192
D = 1024
N = 2
N_CORES = 8
D_LOC = D // N_CORES      # 128 channels per core
J = 16                    # timesteps per block (radix)
C = 8                     # channels per group
G = D_LOC // C            # 16 groups
M = S // J                # 512 blocks per batch
SCALE = math.sqrt(1.0 / N)

F32 = mybir.dt.float32
F16 = mybir.dt.float16


def build_nc(x_bufs=4, o_bufs=3, h_bufs=3, u_bufs=1, y_bufs=1, act="Silu"):
    """Per-core Bass module (SPMD: same NEFF on all cores).

    Inputs (per core):
      x  [B, 128, G*M] f16 — phase-major shard: x[b, j*8+c, g*M+m]
                              = x_orig[b, t=16m+j, ch=8g+c]
      wu [128, G*32]   f16 — u-prep weights, lhsT per group
      wv [128, G*128]  f16 — h-combine weights, lhsT per group
      wx [128, G*128]  f16 — x-combine weights, lhsT per group
      a  [128, 2]      f32 — scan multipliers q^16 for stacked tiles A, B
    Output:
      o  [B, 128, G*M] f16 — same layout as x
    """
    nc = bacc.Bacc(
        "TRN2",
        target_bir_lowering=False,
        debug=False,
        enable_asserts=False,
        num_devices=N_CORES,
    )

    x_d = nc.dram_tensor("x", [B, 128, G * M], F16, kind="ExternalInput").ap()
    wu_d = nc.dram_tensor("wu", [128, G * 32], F16, kind="ExternalInput").ap()
    wv_d = nc.dram_tensor("wv", [128, G * 128], F16, kind="ExternalInput").ap()
    wx_d = nc.dram_tensor("wx", [128, G * 128], F16, kind="ExternalInput").ap()
    a_d = nc.dram_tensor("a", [128, 2], F32, kind="ExternalInput").ap()
    o_d = nc.dram_tensor("o", [B, 128, G * M], F16, kind="ExternalOutput").ap()

    mult = mybir.AluOpType.mult
    add = mybir.AluOpType.add
    ACT = getattr(mybir.ActivationFunctionType, act)
    HALF = G * M // 2

    with tile.TileContext(nc) as tc:
        with (
            tc.tile_pool(name="w", bufs=1) as w_pool,
            tc.tile_pool(name="x", bufs=x_bufs) as x_pool,
            tc.tile_pool(name="o", bufs=o_bufs) as o_pool,
            tc.tile_pool(name="h", bufs=h_bufs) as h_pool,
            tc.tile_pool(name="u", bufs=u_bufs, space="PSUM") as u_pool,
            tc.tile_pool(name="y", bufs=y_bufs, space="PSUM") as y_pool,
        ):
            # param DMAs go FIRST on the Sync ring: DMA-completion semaphore
            # thresholds aggregate per lane in schedule order, so anything
            # issued after a big x transfer inherits its completion time.
            # The scan's `a` multiplier must not wait on x batches.
            wu_t = w_pool.tile([128, G * 32], F16, tag="wu")
            wv_t = w_pool.tile([128, G * 128], F16, tag="wv")
            wx_t = w_pool.tile([128, G * 128], F16, tag="wx")
            a_t = w_pool.tile([128, 2], F32, tag="a")
            # issue order tunes both sem-lane thresholds and arrival time:
            # tiny scan/u params first, then batch-0 x in quarters (so the
            # first u-matmuls start ~5us earlier), then the combine weights
            # (first needed ~15us in), then the remaining batches.
            # tiny scan/u params ride the Scalar ring (issues in parallel
            # with Sync's x0 chunk; nothing big ever queues on Scalar's ring
            # so their completion thresholds clear immediately)
            nc.scalar.dma_start(out=a_t[:], in_=a_d[:])
            nc.scalar.dma_start(out=wu_t[:], in_=wu_d[:])
            x_tiles = [x_pool.tile([128, G * M], F16, tag="x", name=f"x{b}")
                       for b in range(B)]
            QTR = G * M // 4
            # x0's first half in quarters: the tile-A u-matmuls for groups
            # 0-3 start as soon as the first 512KB lands
            nc.sync.dma_start(out=x_tiles[0][:, :QTR], in_=x_d[0, :, :QTR])
            nc.sync.dma_start(
                out=x_tiles[0][:, QTR:HALF], in_=x_d[0, :, QTR:HALF]
            )
            # combine weights land before x0's tail so the first combines
            # start right after scan A of batch 0
            nc.sync.dma_start(out=wv_t[:], in_=wv_d[:])
            nc.sync.dma_start(out=wx_t[:], in_=wx_d[:])
            nc.sync.dma_start(out=x_tiles[0][:, HALF:], in_=x_d[0, :, HALF:])
            # bulk loads in halves: the sweet spot for interleaving with the
            # output ships on the shared DMA queues (2MB and 512KB both
            # measured worse)
            for b in range(1, B):
                nc.sync.dma_start(out=x_tiles[b][:, :HALF], in_=x_d[b, :, :HALF])
                nc.sync.dma_start(out=x_tiles[b][:, HALF:], in_=x_d[b, :, HALF:])

            # PE executes matmuls strictly in emission order, so batch b+1's
            # u-prep must be emitted BETWEEN batch b's combine pairs (once
            # x(b+1) has landed) or the ACT stream stalls at every batch
            # boundary waiting for the next scans.
            u_tiles = {}
            h_tiles = {}

            def emit_u(b, tidx, ks=None):
                """u-prep matmuls for stacked tile tidx (A: groups 0-7)."""
                u_tiles.setdefault(b, [None, None])
                if u_tiles[b][tidx] is None:
                    u_tiles[b][tidx] = u_pool.tile(
                        [128, M], F32, tag=f"u{tidx}", name=f"u{tidx}_{b}"
                    )
                xb = x_tiles[b]
                for k in ks if ks is not None else range(tidx * 4, tidx * 4 + 4):
                    w = k % 4
                    for half in range(2):
                        g = 2 * k + half
                        nc.tensor.matmul(
                            u_tiles[b][tidx][w * 32 : w * 32 + 32, :],
                            lhsT=wu_t[:, g * 32 : (g + 1) * 32],
                            rhs=xb[:, g * M : (g + 1) * M],
                            start=(half == 0),
                            stop=(half == 1),
                            tile_position=(0, w * 32),
                        )

            def emit_scan(b, tidx):
                """h[m] = q^16 h[m-1] + u[m]; fp32 state, fp16 stored h;
                col 0 holds h[-1] = 0 (batches are independent)."""
                h_tiles.setdefault(b, [None, None])
                ht = h_pool.tile(
                    [128, M + 1], F16, tag=f"h{tidx}", name=f"h{tidx}_{b}"
                )
                h_tiles[b][tidx] = ht
                nc.vector.memset(ht[:, 0:1], 0.0)
                nc.vector.tensor_tensor_scan(
                    ht[:, 1 : M + 1],
                    a_t[:, tidx : tidx + 1].broadcast_to([128, M]),
                    u_tiles[b][tidx][:, :],
                    0.0,
                    mult,
                    add,
                )

            def emit_combine(b, g0, ng, ob, y):
                """V+X matmuls for groups [g0, g0+ng) into y, then one ACT."""
                xb = x_tiles[b]
                for i in range(ng):
                    g = g0 + i
                    hv = h_tiles[b][g // 8]
                    ys = y[:, i * M : (i + 1) * M]
                    # x-term first: it only needs xb, so the PE can run it
                    # before the scan (which gates the h-term) completes.
                    nc.tensor.matmul(
                        ys,
                        lhsT=wx_t[:, g * 128 : (g + 1) * 128],
                        rhs=xb[:, g * M : (g + 1) * M],
                        start=True,
                        stop=False,
                    )
                    nc.tensor.matmul(
                        ys,
                        lhsT=wv_t[:, g * 128 : (g + 1) * 128],
                        rhs=hv[:, 0:M],
                        start=False,
                        stop=True,
                    )
                nc.scalar.activation(ob[:, g0 * M : (g0 + ng) * M], y[:], ACT)

            emit_u(0, 0)
            emit_scan(0, 0)
            emit_u(0, 1)
            emit_scan(0, 1)
            # Per batch the 16 groups run as 3|3|3|3|2|2 ACT blocks between
            # two strictly-alternating 3-bank PSUM tags: matmuls fill one
            # tile while ScalarE drains the other, and the bigger FD
            # amortizes the ~300-cycle per-ACT overhead.
            # The 16 groups per batch run as 3|3|2|2|2|2|2 ACT blocks.  The
            # y tags alternate via a GLOBAL counter and the block count is
            # odd, so a block's tag was always freed two ACTs ago, including
            # across batch boundaries.
            BLOCKS = [(0, 3), (3, 3), (6, 2), (8, 2), (10, 2), (12, 2), (14, 2)]
            EIGHTH = G * M // 8
            yslot = 0
            for b in range(B):
                ob = o_pool.tile([128, G * M], F16, tag="o", name=f"o{b}")
                for bi, (g0, ng) in enumerate(BLOCKS):
                    ytag = f"y{yslot % 2}"
                    yslot += 1
                    y = y_pool.tile(
                        [128, 3 * M], F32, tag=ytag, name=f"{ytag}_{b}_{g0}"
                    )
                    emit_combine(b, g0, ng, ob, y[:, : ng * M])
                    # prefetch next batch's u/scans between blocks
                    if b + 1 < B:
                        if bi == 1:
                            emit_u(b + 1, 0)
                            emit_scan(b + 1, 0)
                        elif bi == 3:
                            emit_u(b + 1, 1)
                            emit_scan(b + 1, 1)
                    # ship finished output spans on the Sync ring (ScalarE
                    # stays pure ACT); finer chunks on the last batch to
                    # shrink the drain tail.
                    if b < B - 1:
                        ship = {3: (0, HALF), 6: (HALF, G * M)}.get(bi)
                    else:
                        ship = {1: (0, QTR), 3: (QTR, 2 * QTR),
                                4: (2 * QTR, 3 * QTR),
                                5: (6 * EIGHTH, 7 * EIGHTH),
                                6: (7 * EIGHTH, 8 * EIGHTH)}.get(bi)
                    if ship is not None:
                        nc.sync.dma_start(
                            out=o_d[b, :, ship[0] : ship[1]],
                            in_=ob[:, ship[0] : ship[1]],
                        )

    nc.compile()
    return nc


def _host_params(delta, alpha, beta, gamma, omega, sl):
    """Per-core weight construction (channel slice sl; fp64 math)."""
    d = delta[sl, :, 0].astype(np.float64)
    al = alpha[sl, :, 0].astype(np.float64)
    p = 1.0 / (1.0 + np.exp(-d))
    aa = 1.0 / (1.0 + np.exp(-al))
    q = 1.0 - p * aa                                     # [128, N]
    c = p * beta[sl, :, 0].astype(np.float64) * gamma[sl].astype(np.float64) * SCALE
    w = omega[sl].astype(np.float64)                     # [128]
    ch = np.arange(D_LOC).reshape(G, C)                  # ch[g, cc] = 8g+cc

    qp = q[:, :, None] ** np.arange(J + 2)[None, None, :]   # [128, N, J+2]
    g_r = np.einsum("dn,dnr->dr", c, qp)                 # [128, J+2]

    # stacked-tile partition of (g, n, cc):
    #   tile = g//8, w = (g%8)//2, half = g%2 -> p = 32w + 16*half + 8n + cc
    def stack_p(g, n, cc):
        rem = g % 8
        return 32 * (rem // 2) + 16 * (g % 2) + 8 * n + cc

    wu = np.zeros((G, 128, 32))
    wv = np.zeros((G, 128, 128))
    wx = np.zeros((G, 128, 128))
    a = np.zeros((128, 2))
    for g in range(G):
        for cc in range(C):
            d_ = ch[g, cc]
            # wu cols are window-relative: 16*half + 8n + cc
            col0 = 16 * (g % 2)
            for n in range(N):
                a[stack_p(g, n, cc), g // 8] = qp[d_, n, J]
                for j in range(J):
                    wu[g, j * C + cc, col0 + 8 * n + cc] = qp[d_, n, J - 1 - j]
                    wv[g, stack_p(g, n, cc), j * C + cc] = c[d_, n] * qp[d_, n, j + 1]
            for j in range(J):
                for i in range(j + 1):
                    wx[g, i * C + cc, j * C + cc] = g_r[d_, j - i] + (
                        w[d_] if i == j else 0.0
                    )

    return (
        np.ascontiguousarray(wu.transpose(1, 0, 2).reshape(128, G * 32)).astype(np.float16),
        np.ascontiguousarray(wv.transpose(1, 0, 2).reshape(128, G * 128)).astype(np.float16),
        np.ascontiguousarray(wx.transpose(1, 0, 2).reshape(128, G * 128)).astype(np.float16),
        a.astype(np.float32),
    )


_NC_CACHE = {}


def kernel(x, delta, alpha, beta, gamma, omega):
    x = np.asarray(x, dtype=np.float32)
    delta = np.asarray(delta, dtype=np.float32)
    alpha = np.asarray(alpha, dtype=np.float32)
    beta = np.asarray(beta, dtype=np.float32)
    gamma = np.asarray(gamma, dtype=np.float32)
    omega = np.asarray(omega, dtype=np.float32)
    assert x.shape == (B, S, D)

    if "nc" not in _NC_CACHE:
        _NC_CACHE["nc"] = build_nc()
    nc = _NC_CACHE["nc"]

    xt = x.transpose(0, 2, 1)  # [B, D, S]
    in_maps = []
    for i in range(N_CORES):
        sl = slice(i * D_LOC, (i + 1) * D_LOC)
        wu, wv, wx, a = _host_params(delta, alpha, beta, gamma, omega, sl)
        # phase-major pack: [B, 128ch, S] -> [B, (j,c), g*M+m]
        xs = xt[:, sl, :].reshape(B, G, C, M, J)
        x_dev = np.ascontiguousarray(
            xs.transpose(0, 4, 2, 1, 3).reshape(B, 128, G * M)
        ).astype(np.float16)
        in_maps.append({"x": x_dev, "wu": wu, "wv": wv, "wx": wx, "a": a})

    res = run_bass_kernel_spmd(nc, in_maps, core_ids=list(range(N_CORES)))

    out = np.empty((B, S, D), dtype=np.float32)
    for i in range(N_CORES):
        sl = slice(i * D_LOC, (i + 1) * D_LOC)
        ov = res.results[i]["o"].reshape(B, J, C, G, M)
        oc = ov.transpose(0, 3, 2, 4, 1).reshape(B, D_LOC, S)  # [b, ch, t]
        out[:, :, sl] = oc.transpose(0, 2, 1).astype(np.float32)
    return out
